# revision 1
# baseline (speedup 1.0000x reference)
import os
import sys

for _p in ("/opt/trn_rl_repo", "/root/.axon_site/_ro/trn_rl_repo"):
    if os.path.isdir(_p) and _p not in sys.path:
        sys.path.insert(0, _p)

import numpy as np
from concourse import bacc, tile, mybir
from concourse.bass_utils import run_bass_kernel_spmd

# Problem shapes (hardcoded per spec): x [32,1024,1024], W [3072,1024],
# bias [3072], A0/A1 [5,1024], B0/B1 [1024,5], s0/s1 scalar.
# out [32,1024,3072] = x @ (W + pad(cat(s0*B0@A0, s1*B1@A1)))^T + bias
# Sharding: data-parallel over batch, 4 batches (4096 tokens) per core.
B, S, D = 32, 1024, 1024
O = 3 * D
R = 5
N_CORES = 8
TOK = B * S // N_CORES          # 4096 tokens per core
P = 128
NO = 512                        # output free-dim chunk (one PSUM bank, fp32)
N_D = D // P                    # 8 contraction chunks
N_OC = O // NO                  # 6 output 512-blocks
N_SUP = TOK // NO               # 8 super chunks of 512 tokens
TC = NO // P                    # 4 token tiles per super chunk

F32 = mybir.dt.float32
F32R = mybir.dt.float32r

_CACHE = {}


def _build():
    nc = bacc.Bacc("TRN2", target_bir_lowering=False, debug=False,
                   num_devices=N_CORES)
    x_d = nc.declare_dram_parameter("x", [TOK, D], F32, isOutput=False)
    w_d = nc.declare_dram_parameter("w", [O, D], F32, isOutput=False)
    bias_d = nc.declare_dram_parameter("bias", [1, O], F32, isOutput=False)
    a0_d = nc.declare_dram_parameter("a0", [R, D], F32, isOutput=False)
    a1_d = nc.declare_dram_parameter("a1", [R, D], F32, isOutput=False)
    b0_d = nc.declare_dram_parameter("b0", [D, R], F32, isOutput=False)
    b1_d = nc.declare_dram_parameter("b1", [D, R], F32, isOutput=False)
    s_d = nc.declare_dram_parameter("svec", [1, 2], F32, isOutput=False)
    ident_d = nc.declare_dram_parameter("ident", [P, P], F32, isOutput=False)
    ones_d = nc.declare_dram_parameter("ones", [1, P], F32, isOutput=False)
    out_d = nc.declare_dram_parameter("out", [TOK, O], F32, isOutput=True)

    ADD = mybir.AluOpType.add
    MUL = mybir.AluOpType.mult

    with tile.TileContext(nc) as tc:
        with tc.tile_pool(name="const", bufs=1) as cpool, \
             tc.tile_pool(name="wres", bufs=1) as wpool, \
             tc.tile_pool(name="xload", bufs=7) as xpool, \
             tc.tile_pool(name="xt", bufs=2) as xtpool, \
             tc.tile_pool(name="ostage", bufs=4) as opool, \
             tc.tile_pool(name="psA", bufs=4, space="PSUM") as psA, \
             tc.tile_pool(name="psT", bufs=4, space="PSUM") as psT:

            # ---- constants ----
            ident_sb = cpool.tile([P, P], F32R, tag="ident")
            nc.sync.dma_start(out=ident_sb[:], in_=ident_d[:].bitcast(F32R))
            ones_sb = cpool.tile([1, P], F32, tag="ones")
            nc.sync.dma_start(out=ones_sb[:], in_=ones_d[:])
            bias1_sb = cpool.tile([1, O], F32, tag="bias1")
            nc.sync.dma_start(out=bias1_sb[:], in_=bias_d[:])
            a_sb = []
            for i, ad in enumerate((a0_d, a1_d)):
                t = cpool.tile([R, D], F32R, tag=f"a{i}", name=f"a{i}")
                nc.sync.dma_start(out=t[:], in_=ad[:].bitcast(F32R))
                a_sb.append(t)
            bt_sb = []
            for i, bd in enumerate((b0_d, b1_d)):
                t = cpool.tile([R, D], F32R, tag=f"bt{i}", name=f"bt{i}")
                nc.sync.dma_start(out=t[:], in_=bd.rearrange("k r -> r k").bitcast(F32R))
                bt_sb.append(t)
            s1_sb = cpool.tile([1, 2], F32, tag="s1")
            nc.sync.dma_start(out=s1_sb[:], in_=s_d[:])

            # broadcast s to all partitions via K=1 matmul with ones
            s_ps = psT.tile([P, 2], F32, tag="tp")
            nc.tensor.matmul(s_ps[:], ones_sb[:], s1_sb[:], start=True, stop=True)
            s_bc = cpool.tile([P, 2], F32, tag="sbc")
            nc.vector.tensor_copy(s_bc[:], s_ps[:])

            # fold s0/s1 into the B^T factors (in-place, stays F32R)
            for i in range(2):
                nc.vector.tensor_scalar(out=bt_sb[i][:], in0=bt_sb[i][:].bitcast(F32),
                                        scalar1=s_bc[0:R, i:i + 1], scalar2=None,
                                        op0=MUL)

            # broadcast bias across partitions: [128, 3072]
            bias_bc = cpool.tile([P, O], F32, tag="biasbc")
            for j in range(N_OC):
                sl = slice(j * NO, (j + 1) * NO)
                b_ps = psA.tile([P, NO], F32, tag="acc")
                nc.tensor.matmul(b_ps[:], ones_sb[:], bias1_sb[:, sl],
                                 start=True, stop=True)
                nc.vector.tensor_copy(bias_bc[:, sl], b_ps[:])

            # ---- resident W'^T, 48 tiles [128, 512]: wt[d][ocb] ----
            # Per (ocb, d): 4 PE transposes fill one PSUM bank [128,512];
            # for LoRA blocks one accumulating rank-5 matmul adds the delta;
            # a single DVE copy drains to SBUF (fp32r).
            wt = [[wpool.tile([P, NO], F32R, tag=f"wt{d}_{ocb}",
                              name=f"wt{d}_{ocb}")
                   for ocb in range(N_OC)] for d in range(N_D)]
            # ---- main loop: 8 super-chunks of 512 tokens ----
            def emit_super_loads(sp):
                x_nat = []
                for tci in range(TC):
                    row0 = sp * NO + tci * P
                    xn = xpool.tile([P, D], F32R, tag="xnat", name=f"xn{sp}_{tci}")
                    nc.scalar.dma_start(out=xn[:],
                                        in_=x_d[row0:row0 + P, :].bitcast(F32R))
                    x_nat.append(xn)
                xg = [[None, None] for _ in range(TC)]
                for tci in range(TC):
                    for g in range(2):
                        tp = psT.tile([P, NO], F32R, tag="tp", name="tp")
                        for k in range(4):
                            d = g * 4 + k
                            nc.tensor.matmul(tp[:, k * P:(k + 1) * P],
                                             x_nat[tci][:, d * P:(d + 1) * P],
                                             ident_sb[:], is_transpose=True,
                                             start=(k == 0), stop=(k == 3),
                                             skip_group_check=True)
                        xgt = xtpool.tile([P, NO], F32R, tag=f"xg{tci}_{g}",
                                          name=f"xg{tci}_{g}")
                        nc.vector.tensor_copy(xgt[:], tp[:].bitcast(F32))
                        xg[tci][g] = xgt
                return xg

            def emit_super_mms(sp, xg):
                for tci in range(TC):
                    trow = slice(sp * NO + tci * P, sp * NO + (tci + 1) * P)
                    for oc in range(N_OC):
                        osl = slice(oc * NO, (oc + 1) * NO)
                        acc = psA.tile([P, NO], F32, tag="acc", name="acc")
                        for d in range(N_D):
                            lhsT = xg[tci][d // 4][:, (d % 4) * P:(d % 4 + 1) * P]
                            nc.tensor.matmul(acc[:], lhsT, wt[d][oc][:],
                                             start=(d == 0), stop=(d == N_D - 1))
                        o_sb = opool.tile([P, NO], F32, tag="ost", name="ost")
                        nc.vector.tensor_tensor(out=o_sb[:], in0=acc[:],
                                                in1=bias_bc[:, osl], op=ADD)
                        nc.sync.dma_start(out=out_d[trow, osl], in_=o_sb[:])

            xg_pending = {sp: emit_super_loads(sp) for sp in range(2)}
            for ocb in range(N_OC):
                w_nat = []
                for j in range(TC):
                    oc = ocb * TC + j
                    wn = xpool.tile([P, D], F32R, tag="xnat", name=f"wn{oc}")
                    nc.scalar.dma_start(out=wn[:],
                                      in_=w_d[oc * P:(oc + 1) * P, :].bitcast(F32R))
                    w_nat.append(wn)
                for d in range(N_D):
                    tp = psT.tile([P, NO], F32R, tag="tp")
                    for j in range(TC):
                        nc.tensor.matmul(tp[:, j * P:(j + 1) * P],
                                         w_nat[j][:, d * P:(d + 1) * P],
                                         ident_sb[:], is_transpose=True,
                                         start=(j == 0),
                                         stop=(j == TC - 1 and ocb < 2),
                                         skip_group_check=True)
                    if ocb >= 2:
                        f = 0 if ocb < 4 else 1
                        lo = ocb * NO - D - (D if f else 0)
                        nc.tensor.matmul(tp[:].bitcast(F32),
                                         a_sb[f][:, d * P:(d + 1) * P],
                                         bt_sb[f][:, lo:lo + NO],
                                         start=False, stop=True,
                                         skip_group_check=True)
                    nc.vector.tensor_copy(wt[d][ocb][:], tp[:].bitcast(F32))

            for sp in range(N_SUP):
                if sp not in xg_pending:
                    xg_pending[sp] = emit_super_loads(sp)
                emit_super_mms(sp, xg_pending.pop(sp))

    nc.compile()
    return nc


def kernel(x, W, bias, A0, A1, B0, B1, s0, s1, **run_kwargs):
    x = np.asarray(x, dtype=np.float32)
    if "nc" not in _CACHE:
        _CACHE["nc"] = _build()
    nc = _CACHE["nc"]

    shared = {
        "w": np.ascontiguousarray(np.asarray(W, np.float32)),
        "bias": np.asarray(bias, np.float32).reshape(1, O),
        "a0": np.ascontiguousarray(np.asarray(A0, np.float32)),
        "a1": np.ascontiguousarray(np.asarray(A1, np.float32)),
        "b0": np.ascontiguousarray(np.asarray(B0, np.float32)),
        "b1": np.ascontiguousarray(np.asarray(B1, np.float32)),
        "svec": np.array([[np.float32(s0), np.float32(s1)]], np.float32),
        "ident": np.eye(P, dtype=np.float32),
        "ones": np.ones((1, P), np.float32),
    }
    xr = x.reshape(N_CORES, TOK, D)
    in_maps = [{**shared, "x": np.ascontiguousarray(xr[c])} for c in range(N_CORES)]
    res = run_bass_kernel_spmd(nc, in_maps, list(range(N_CORES)), **run_kwargs)
    out = np.concatenate([res.results[c]["out"][None] for c in range(N_CORES)], 0)
    full = out.reshape(B, S, O)
    _CACHE["last_result"] = res
    return full



# revision 2
# speedup vs baseline: 1.0687x; 1.0687x over previous
import os
import sys

for _p in ("/opt/trn_rl_repo", "/root/.axon_site/_ro/trn_rl_repo"):
    if os.path.isdir(_p) and _p not in sys.path:
        sys.path.insert(0, _p)

import numpy as np
from concourse import bacc, tile, mybir
from concourse.bass_utils import run_bass_kernel_spmd

# Problem shapes (hardcoded per spec): x [32,1024,1024], W [3072,1024],
# bias [3072], A0/A1 [5,1024], B0/B1 [1024,5], s0/s1 scalar.
# out [32,1024,3072] = x @ (W + pad(cat(s0*B0@A0, s1*B1@A1)))^T + bias
# Sharding: data-parallel over batch, 4 batches (4096 tokens) per core.
#
# Per-core structure:
#   - Build W'^T = (W + delta)^T resident in SBUF as 48 bf16 tiles [128,512]
#     (PE transposes in fp32r, LoRA rank-5 accumulated in fp32 PSUM, single
#     bf16 round on the DVE drain).
#   - x is PE-transposed per 512-token superchunk (fp32r), drained to bf16.
#   - Main matmuls run bf16 x bf16 -> fp32 PSUM at 1 cycle/row.
#   - W prep is interleaved per-ocb with the first two superchunks' matmuls
#     so the PE never sits idle waiting on the 12 MB W DMA.
#   - DMA queues: x on scalar (HWDGE), W on gpsimd (SWDGE), consts + output
#     stores on sync (HWDGE).
B, S, D = 32, 1024, 1024
O = 3 * D
R = 5
N_CORES = 8
TOK = B * S // N_CORES          # 4096 tokens per core
P = 128
NO = 512                        # output free-dim chunk (one PSUM bank, fp32)
N_D = D // P                    # 8 contraction chunks
N_OC = O // NO                  # 6 output 512-blocks
N_SUP = TOK // NO               # 8 super chunks of 512 tokens
TC = NO // P                    # 4 token tiles per super chunk

F32 = mybir.dt.float32
F32R = mybir.dt.float32r
BF16 = mybir.dt.bfloat16

_CACHE = {}


def _build():
    nc = bacc.Bacc("TRN2", target_bir_lowering=False, debug=False,
                   num_devices=N_CORES)
    x_d = nc.declare_dram_parameter("x", [TOK, D], F32, isOutput=False)
    w_d = nc.declare_dram_parameter("w", [O, D], F32, isOutput=False)
    bias_d = nc.declare_dram_parameter("bias", [1, O], F32, isOutput=False)
    a0_d = nc.declare_dram_parameter("a0", [R, D], F32, isOutput=False)
    a1_d = nc.declare_dram_parameter("a1", [R, D], F32, isOutput=False)
    b0_d = nc.declare_dram_parameter("b0", [D, R], F32, isOutput=False)
    b1_d = nc.declare_dram_parameter("b1", [D, R], F32, isOutput=False)
    s_d = nc.declare_dram_parameter("svec", [1, 2], F32, isOutput=False)
    ident_d = nc.declare_dram_parameter("ident", [P, P], F32, isOutput=False)
    ones_d = nc.declare_dram_parameter("ones", [1, P], F32, isOutput=False)
    out_d = nc.declare_dram_parameter("out", [TOK, O], F32, isOutput=True)

    ADD = mybir.AluOpType.add
    MUL = mybir.AluOpType.mult

    with tile.TileContext(nc) as tc:
        with tc.tile_pool(name="const", bufs=1) as cpool, \
             tc.tile_pool(name="wres", bufs=1) as wpool, \
             tc.tile_pool(name="xload", bufs=8) as xpool, \
             tc.tile_pool(name="wload", bufs=8) as wnpool, \
             tc.tile_pool(name="xt", bufs=2) as xtpool, \
             tc.tile_pool(name="ostage", bufs=4) as opool, \
             tc.tile_pool(name="psA", bufs=4, space="PSUM") as psA, \
             tc.tile_pool(name="psT", bufs=4, space="PSUM") as psT:

            # ---- const DMAs (sync queue; ident first: transposes need it) ----
            ident_sb = cpool.tile([P, P], F32R, tag="ident")
            nc.sync.dma_start(out=ident_sb[:], in_=ident_d[:].bitcast(F32R))

            # ---- x superchunk 0/1 loads (scalar queue) ----
            def emit_x_loads(sp):
                x_nat = []
                for tci in range(TC):
                    row0 = sp * NO + tci * P
                    xn = xpool.tile([P, D], F32R, tag="xnat", name=f"xn{sp}_{tci}")
                    nc.scalar.dma_start(out=xn[:],
                                        in_=x_d[row0:row0 + P, :].bitcast(F32R))
                    x_nat.append(xn)
                return x_nat

            x_nat_pending = {0: emit_x_loads(0), 1: emit_x_loads(1)}

            # remaining consts on sync queue (small; b0/b1 are strided gathers)
            ones_sb = cpool.tile([1, P], F32, tag="ones")
            nc.sync.dma_start(out=ones_sb[:], in_=ones_d[:])
            bias1_sb = cpool.tile([1, O], F32, tag="bias1")
            nc.sync.dma_start(out=bias1_sb[:], in_=bias_d[:])
            s1_sb = cpool.tile([1, 2], F32, tag="s1")
            nc.sync.dma_start(out=s1_sb[:], in_=s_d[:])
            a_sb = []
            for i, ad in enumerate((a0_d, a1_d)):
                t = cpool.tile([R, D], F32R, tag=f"a{i}", name=f"a{i}")
                nc.sync.dma_start(out=t[:], in_=ad[:].bitcast(F32R))
                a_sb.append(t)
            bt_sb = []
            for i, bd in enumerate((b0_d, b1_d)):
                t = cpool.tile([R, D], F32R, tag=f"bt{i}", name=f"bt{i}")
                nc.sync.dma_start(out=t[:], in_=bd.rearrange("k r -> r k").bitcast(F32R))
                bt_sb.append(t)

            # ---- W loads (gpsimd queue, all 24 tiles; ring-buffered) ----
            w_nat = {}
            for ocb in range(N_OC):
                tiles = []
                for j in range(TC):
                    oc = ocb * TC + j
                    wn = wnpool.tile([P, D], F32R, tag="wnat", name=f"wn{oc}")
                    nc.gpsimd.dma_start(out=wn[:],
                                        in_=w_d[oc * P:(oc + 1) * P, :].bitcast(F32R))
                    tiles.append(wn)
                w_nat[ocb] = tiles

            # ---- x transpose per superchunk: fp32r PE transpose, bf16 drain ----
            def emit_x_transposes(sp, x_nat):
                xg = [[None, None] for _ in range(TC)]
                for tci in range(TC):
                    for g in range(2):
                        tp = psT.tile([P, NO], F32R, tag="tp", name="tp")
                        for k in range(4):
                            d = g * 4 + k
                            nc.tensor.matmul(tp[:, k * P:(k + 1) * P],
                                             x_nat[tci][:, d * P:(d + 1) * P],
                                             ident_sb[:], is_transpose=True,
                                             start=(k == 0), stop=(k == 3),
                                             skip_group_check=True)
                        xgt = xtpool.tile([P, NO], BF16, tag=f"xg{tci}_{g}",
                                          name=f"xg{tci}_{g}")
                        nc.vector.tensor_copy(xgt[:], tp[:].bitcast(F32))
                        xg[tci][g] = xgt
                return xg

            xg_pending = {sp: emit_x_transposes(sp, x_nat_pending.pop(sp))
                          for sp in range(2)}

            # ---- s broadcast + fold into B^T factors; bias broadcast ----
            s_ps = psT.tile([P, 2], F32, tag="tp")
            nc.tensor.matmul(s_ps[:], ones_sb[:], s1_sb[:], start=True, stop=True)
            s_bc = cpool.tile([P, 2], F32, tag="sbc")
            nc.vector.tensor_copy(s_bc[:], s_ps[:])
            for i in range(2):
                nc.vector.tensor_scalar(out=bt_sb[i][:], in0=bt_sb[i][:].bitcast(F32),
                                        scalar1=s_bc[0:R, i:i + 1], scalar2=None,
                                        op0=MUL)

            bias_bc = cpool.tile([P, O], F32, tag="biasbc")
            for j in range(N_OC):
                sl = slice(j * NO, (j + 1) * NO)
                b_ps = psA.tile([P, NO], F32, tag="acc")
                nc.tensor.matmul(b_ps[:], ones_sb[:], bias1_sb[:, sl],
                                 start=True, stop=True)
                nc.vector.tensor_copy(bias_bc[:, sl], b_ps[:])

            # ---- resident W'^T, 48 bf16 tiles [128, 512]: wt[d][ocb] ----
            wt = [[wpool.tile([P, NO], BF16, tag=f"wt{d}_{ocb}",
                              name=f"wt{d}_{ocb}")
                   for ocb in range(N_OC)] for d in range(N_D)]

            def emit_w_prep(ocb):
                for d in range(N_D):
                    tp = psT.tile([P, NO], F32R, tag="tp")
                    for j in range(TC):
                        nc.tensor.matmul(tp[:, j * P:(j + 1) * P],
                                         w_nat[ocb][j][:, d * P:(d + 1) * P],
                                         ident_sb[:], is_transpose=True,
                                         start=(j == 0),
                                         stop=(j == TC - 1 and ocb < 2),
                                         skip_group_check=True)
                    if ocb >= 2:
                        f = 0 if ocb < 4 else 1
                        lo = ocb * NO - D - (D if f else 0)
                        nc.tensor.matmul(tp[:].bitcast(F32),
                                         a_sb[f][:, d * P:(d + 1) * P],
                                         bt_sb[f][:, lo:lo + NO],
                                         start=False, stop=True,
                                         skip_group_check=True)
                    nc.vector.tensor_copy(wt[d][ocb][:], tp[:].bitcast(F32))

            # ---- main matmuls for one superchunk, restricted to ocb list ----
            def emit_super_mms(sp, xg, ocbs):
                for tci in range(TC):
                    trow = slice(sp * NO + tci * P, sp * NO + (tci + 1) * P)
                    for oc in ocbs:
                        osl = slice(oc * NO, (oc + 1) * NO)
                        acc = psA.tile([P, NO], F32, tag="acc", name="acc")
                        for d in range(N_D):
                            lhsT = xg[tci][d // 4][:, (d % 4) * P:(d % 4 + 1) * P]
                            nc.tensor.matmul(acc[:], lhsT, wt[d][oc][:],
                                             start=(d == 0), stop=(d == N_D - 1))
                        o_sb = opool.tile([P, NO], F32, tag="ost", name="ost")
                        nc.vector.tensor_tensor(out=o_sb[:], in0=acc[:],
                                                in1=bias_bc[:, osl], op=ADD)
                        nc.sync.dma_start(out=out_d[trow, osl], in_=o_sb[:])

            # ---- W prep interleaved with sp0/sp1 matmuls, one ocb at a time ----
            for ocb in range(N_OC):
                emit_w_prep(ocb)
                emit_super_mms(0, xg_pending[0], [ocb])
                emit_super_mms(1, xg_pending[1], [ocb])

            # ---- steady state: superchunks 2..7 ----
            for sp in range(2, N_SUP):
                x_nat = emit_x_loads(sp)
                xg = emit_x_transposes(sp, x_nat)
                emit_super_mms(sp, xg, range(N_OC))

    nc.compile()
    return nc


def kernel(x, W, bias, A0, A1, B0, B1, s0, s1, **run_kwargs):
    x = np.asarray(x, dtype=np.float32)
    if "nc" not in _CACHE:
        _CACHE["nc"] = _build()
    nc = _CACHE["nc"]

    shared = {
        "w": np.ascontiguousarray(np.asarray(W, np.float32)),
        "bias": np.asarray(bias, np.float32).reshape(1, O),
        "a0": np.ascontiguousarray(np.asarray(A0, np.float32)),
        "a1": np.ascontiguousarray(np.asarray(A1, np.float32)),
        "b0": np.ascontiguousarray(np.asarray(B0, np.float32)),
        "b1": np.ascontiguousarray(np.asarray(B1, np.float32)),
        "svec": np.array([[np.float32(s0), np.float32(s1)]], np.float32),
        "ident": np.eye(P, dtype=np.float32),
        "ones": np.ones((1, P), np.float32),
    }
    xr = x.reshape(N_CORES, TOK, D)
    in_maps = [{**shared, "x": np.ascontiguousarray(xr[c])} for c in range(N_CORES)]
    res = run_bass_kernel_spmd(nc, in_maps, list(range(N_CORES)), **run_kwargs)
    out = np.concatenate([res.results[c]["out"][None] for c in range(N_CORES)], 0)
    full = out.reshape(B, S, O)
    _CACHE["last_result"] = res
    return full


# revision 3
# speedup vs baseline: 1.1885x; 1.1120x over previous
import os
import sys

for _p in ("/opt/trn_rl_repo", "/root/.axon_site/_ro/trn_rl_repo"):
    if os.path.isdir(_p) and _p not in sys.path:
        sys.path.insert(0, _p)

import numpy as np
from concourse import bacc, tile, mybir
from concourse.bass_utils import run_bass_kernel_spmd

# Problem shapes (hardcoded per spec): x [32,1024,1024], W [3072,1024],
# bias [3072], A0/A1 [5,1024], B0/B1 [1024,5], s0/s1 scalar.
# out [32,1024,3072] = x @ (W + pad(cat(s0*B0@A0, s1*B1@A1)))^T + bias
# Sharding: data-parallel over batch, 4 batches (4096 tokens) per core.
#
# Per-core structure:
#   - W'^T = (W + delta)^T resident in SBUF as 48 bf16 tiles [128,512]
#     (PE transposes in fp32r, LoRA rank-5 accumulated in fp32 PSUM from
#     replicated A / (s*B)^T factors, single bf16 round on the DVE drain).
#   - x is PE-transposed per 512-token superchunk (fp32r), drained to bf16.
#   - Main matmuls run bf16 x bf16 -> fp32 PSUM at 1 cycle/row.
#   - W prep PSUM groups are interleaved 1:1 with the first two superchunks'
#     accumulation groups, keeping PE MAC activity high (HAM clock stays at
#     2.4 GHz) and hiding the 12 MB W DMA.
#   - DMA queues: x on scalar (HWDGE), W on gpsimd (SWDGE), consts + output
#     stores on sync (HWDGE).
#   - Host-side marshalling: (s*B).T is precomputed on host (tiny [1024,5]
#     tensors; avoids a pathological 4-byte-strided gather DMA on device).
B, S, D = 32, 1024, 1024
O = 3 * D
R = 5
N_CORES = 8
TOK = B * S // N_CORES          # 4096 tokens per core
P = 128
NO = 512                        # output free-dim chunk (one PSUM bank, fp32)
N_D = D // P                    # 8 contraction chunks
N_OC = O // NO                  # 6 output 512-blocks
N_SUP = TOK // NO               # 8 super chunks of 512 tokens
TC = NO // P                    # 4 token tiles per super chunk

F32 = mybir.dt.float32
F32R = mybir.dt.float32r
BF16 = mybir.dt.bfloat16

_CACHE = {}


def _build():
    nc = bacc.Bacc("TRN2", target_bir_lowering=False, debug=False,
                   num_devices=N_CORES)
    x_d = nc.declare_dram_parameter("x", [TOK, D], F32, isOutput=False)
    w_d = nc.declare_dram_parameter("w", [O, D], F32, isOutput=False)
    bias_d = nc.declare_dram_parameter("bias", [1, O], F32, isOutput=False)
    a0_d = nc.declare_dram_parameter("a0", [R, D], F32, isOutput=False)
    a1_d = nc.declare_dram_parameter("a1", [R, D], F32, isOutput=False)
    bt0_d = nc.declare_dram_parameter("bt0", [R, D], F32, isOutput=False)
    bt1_d = nc.declare_dram_parameter("bt1", [R, D], F32, isOutput=False)
    ident_d = nc.declare_dram_parameter("ident", [P, P], F32, isOutput=False)
    ones_d = nc.declare_dram_parameter("ones", [1, P], F32, isOutput=False)
    out_d = nc.declare_dram_parameter("out", [TOK, O], F32, isOutput=True)

    ADD = mybir.AluOpType.add

    with tile.TileContext(nc) as tc:
        with tc.tile_pool(name="const", bufs=1) as cpool, \
             tc.tile_pool(name="wres", bufs=1) as wpool, \
             tc.tile_pool(name="xload", bufs=8) as xpool, \
             tc.tile_pool(name="wload", bufs=8) as wnpool, \
             tc.tile_pool(name="xt", bufs=2) as xtpool, \
             tc.tile_pool(name="ostage", bufs=4) as opool, \
             tc.tile_pool(name="psA", bufs=4, space="PSUM") as psA, \
             tc.tile_pool(name="psT", bufs=4, space="PSUM") as psT:

            # ---- const DMAs (sync queue; ident first: transposes need it) ----
            ident_sb = cpool.tile([P, P], F32R, tag="ident")
            nc.sync.dma_start(out=ident_sb[:], in_=ident_d[:].bitcast(F32R))

            # ---- x superchunk 0/1 loads (scalar queue) ----
            def emit_x_loads(sp):
                x_nat = []
                for tci in range(TC):
                    row0 = sp * NO + tci * P
                    xn = xpool.tile([P, D], F32R, tag="xnat", name=f"xn{sp}_{tci}")
                    nc.scalar.dma_start(out=xn[:],
                                        in_=x_d[row0:row0 + P, :].bitcast(F32R))
                    x_nat.append(xn)
                return x_nat

            x_nat_pending = {0: emit_x_loads(0), 1: emit_x_loads(1)}

            # remaining consts on sync queue (all small + natural layout)
            ones_sb = cpool.tile([1, P], F32, tag="ones")
            nc.sync.dma_start(out=ones_sb[:], in_=ones_d[:])
            bias1_sb = cpool.tile([1, O], F32, tag="bias1")
            nc.sync.dma_start(out=bias1_sb[:], in_=bias_d[:])
            a_sb = []
            for i, ad in enumerate((a0_d, a1_d)):
                t = cpool.tile([R, D], F32R, tag=f"a{i}", name=f"a{i}")
                nc.sync.dma_start(out=t[:], in_=ad[:].bitcast(F32R))
                a_sb.append(t)
            bt_sb = []
            for i, bd in enumerate((bt0_d, bt1_d)):
                t = cpool.tile([R, D], F32R, tag=f"bt{i}", name=f"bt{i}")
                nc.sync.dma_start(out=t[:], in_=bd[:].bitcast(F32R))
                bt_sb.append(t)

            # ---- W loads (gpsimd queue, all 24 tiles; ring-buffered) ----
            w_nat = {}
            for ocb in range(N_OC):
                tiles = []
                for j in range(TC):
                    oc = ocb * TC + j
                    wn = wnpool.tile([P, D], F32R, tag="wnat", name=f"wn{oc}")
                    nc.gpsimd.dma_start(out=wn[:],
                                        in_=w_d[oc * P:(oc + 1) * P, :].bitcast(F32R))
                    tiles.append(wn)
                w_nat[ocb] = tiles

            # ---- x transpose per superchunk: fp32r PE transpose, bf16 drain ----
            def emit_x_transposes(sp, x_nat):
                xg = [[None, None] for _ in range(TC)]
                for tci in range(TC):
                    for g in range(2):
                        tp = psT.tile([P, NO], F32R, tag="tp", name="tp")
                        for k in range(4):
                            d = g * 4 + k
                            nc.tensor.matmul(tp[:, k * P:(k + 1) * P],
                                             x_nat[tci][:, d * P:(d + 1) * P],
                                             ident_sb[:], is_transpose=True,
                                             start=(k == 0), stop=(k == 3),
                                             skip_group_check=True)
                        xgt = xtpool.tile([P, NO], BF16, tag=f"xg{tci}_{g}",
                                          name=f"xg{tci}_{g}")
                        nc.vector.tensor_copy(xgt[:], tp[:].bitcast(F32))
                        xg[tci][g] = xgt
                return xg

            xg_pending = {sp: emit_x_transposes(sp, x_nat_pending.pop(sp))
                          for sp in range(2)}

            # ---- bias broadcast across partitions: [128, 3072] ----
            bias_bc = cpool.tile([P, O], F32, tag="biasbc")
            for j in range(N_OC):
                sl = slice(j * NO, (j + 1) * NO)
                b_ps = psA.tile([P, NO], F32, tag="acc")
                nc.tensor.matmul(b_ps[:], ones_sb[:], bias1_sb[:, sl],
                                 start=True, stop=True)
                nc.vector.tensor_copy(bias_bc[:, sl], b_ps[:])

            # ---- resident W'^T, 48 bf16 tiles [128, 512]: wt[d][ocb] ----
            wt = [[wpool.tile([P, NO], BF16, tag=f"wt{d}_{ocb}",
                              name=f"wt{d}_{ocb}")
                   for ocb in range(N_OC)] for d in range(N_D)]

            def emit_w_prep_group(ocb, d):
                # one PSUM group: 4 PE transposes (+ LoRA accumulate) + drain
                tp = psT.tile([P, NO], F32R, tag="tp")
                for j in range(TC):
                    nc.tensor.matmul(tp[:, j * P:(j + 1) * P],
                                     w_nat[ocb][j][:, d * P:(d + 1) * P],
                                     ident_sb[:], is_transpose=True,
                                     start=(j == 0),
                                     stop=(j == TC - 1 and ocb < 2),
                                     skip_group_check=True)
                if ocb >= 2:
                    f = 0 if ocb < 4 else 1
                    lo = ocb * NO - D - (D if f else 0)
                    nc.tensor.matmul(tp[:].bitcast(F32),
                                     a_sb[f][:, d * P:(d + 1) * P],
                                     bt_sb[f][:, lo:lo + NO],
                                     start=False, stop=True,
                                     skip_group_check=True)
                nc.vector.tensor_copy(wt[d][ocb][:], tp[:].bitcast(F32))

            # ---- one accumulation group of main matmuls + drain + store ----
            def emit_acc_group(sp, tci, oc, xg):
                trow = slice(sp * NO + tci * P, sp * NO + (tci + 1) * P)
                osl = slice(oc * NO, (oc + 1) * NO)
                acc = psA.tile([P, NO], F32, tag="acc", name="acc")
                for d in range(N_D):
                    lhsT = xg[tci][d // 4][:, (d % 4) * P:(d % 4 + 1) * P]
                    nc.tensor.matmul(acc[:], lhsT, wt[d][oc][:],
                                     start=(d == 0), stop=(d == N_D - 1))
                o_sb = opool.tile([P, NO], F32, tag="ost", name="ost")
                nc.vector.tensor_tensor(out=o_sb[:], in0=acc[:],
                                        in1=bias_bc[:, osl], op=ADD)
                nc.sync.dma_start(out=out_d[trow, osl], in_=o_sb[:])

            # ---- startup: W prep interleaved with sp0/sp1 matmuls ----
            # prep(0) first, then for each ocb: 8 acc groups (sp0/sp1 x 4 tci)
            # interleaved 1:1 with the 8 prep groups of ocb+1.
            for d in range(N_D):
                emit_w_prep_group(0, d)
            for ocb in range(N_OC):
                groups = [(sp, tci) for sp in (0, 1) for tci in range(TC)]
                for i, (sp, tci) in enumerate(groups):
                    emit_acc_group(sp, tci, ocb, xg_pending[sp])
                    if ocb + 1 < N_OC:
                        emit_w_prep_group(ocb + 1, i)

            # ---- steady state: superchunks 2..7 ----
            for sp in range(2, N_SUP):
                x_nat = emit_x_loads(sp)
                xg = emit_x_transposes(sp, x_nat)
                for tci in range(TC):
                    for oc in range(N_OC):
                        emit_acc_group(sp, tci, oc, xg)

    nc.compile()
    return nc


def kernel(x, W, bias, A0, A1, B0, B1, s0, s1, **run_kwargs):
    x = np.asarray(x, dtype=np.float32)
    if "nc" not in _CACHE:
        _CACHE["nc"] = _build()
    nc = _CACHE["nc"]

    s0 = np.float32(np.asarray(s0).reshape(()))
    s1 = np.float32(np.asarray(s1).reshape(()))
    shared = {
        "w": np.ascontiguousarray(np.asarray(W, np.float32)),
        "bias": np.asarray(bias, np.float32).reshape(1, O),
        "a0": np.ascontiguousarray(np.asarray(A0, np.float32)),
        "a1": np.ascontiguousarray(np.asarray(A1, np.float32)),
        "bt0": np.ascontiguousarray((s0 * np.asarray(B0, np.float32)).T),
        "bt1": np.ascontiguousarray((s1 * np.asarray(B1, np.float32)).T),
        "ident": np.eye(P, dtype=np.float32),
        "ones": np.ones((1, P), np.float32),
    }
    xr = x.reshape(N_CORES, TOK, D)
    in_maps = [{**shared, "x": np.ascontiguousarray(xr[c])} for c in range(N_CORES)]
    res = run_bass_kernel_spmd(nc, in_maps, list(range(N_CORES)), **run_kwargs)
    out = np.concatenate([res.results[c]["out"][None] for c in range(N_CORES)], 0)
    full = out.reshape(B, S, O)
    _CACHE["last_result"] = res
    return full
